# revision 17
# baseline (speedup 1.0000x reference)
"""Two-layer GAT (GATConv heads=1, PyG-style) on 8 Trainium2 NeuronCores.

Strategy (matches sharding_hint): nodes are degree-sorted and dealt
round-robin to the 8 cores (so every core sees the same degree profile,
letting one SPMD program with a fixed per-group ELL K-schedule serve all
cores). Edges are partitioned by destination node. Each core computes the
feature transform for its own node slice, the per-node tables
[h | alpha_src (| alpha_dst)] are AllGathered so each core holds the full
table in local HBM, and edge aggregation is done with multi-index indirect
DMA gathers (ELL layout: node = SBUF partition, edge slot = free dim)
followed by a segment softmax + weighted reduction on the vector/scalar
engines. Layer 2 repeats the pattern with a second AllGather.
"""
import sys

sys.path.insert(0, "/opt/trn_rl_repo")

import numpy as np

# ---------------------------------------------------------------- constants
N = 50000        # nodes
F_IN = 512       # input features
HID = 64         # layer-1 out features
NCLS = 40        # classes
NEG_SLOPE = 0.2

NCORES = 8
P = 128                       # SBUF partitions
NG = 49                       # node groups per core
NPC = NG * P                  # node slots per core (6272; 6250 real)
NREAL = N // NCORES           # real nodes per core (6250)
DEBUG_DUMPS = False           # add intermediate ExternalOutputs (debug only)
NPC1 = NPC + 1                # slice rows incl per-core dummy row
NT = NCORES * NPC1            # global permuted table rows (50184)
DUMMY = NPC                   # dummy row index = core-0's dummy slot
ALPHA_PAD = -60.0

R1 = HID + 2                  # table1 row: [h(64) | a_src | a_dst] = 66
R2 = 44                       # table2 row: [h2(40) | a_src | a_dst | pad] = 44

F32 = None  # filled after imports


def _import_bass():
    global bass, bacc, mybir, tile, F32
    import concourse.bass as bass
    import concourse.bacc as bacc
    import concourse.mybir as mybir
    import concourse.tile as tile
    F32 = mybir.dt.float32
    return bass, mybir, tile


# ---------------------------------------------------------------- host prep
def preprocess(x, edge_index, W1, a_src1, a_dst1, b1, W2, a_src2, a_dst2, b2):
    """Degree-sort nodes, deal round-robin to cores, build ELL edge arrays."""
    src = np.asarray(edge_index[0], dtype=np.int64)
    dst = np.asarray(edge_index[1], dtype=np.int64)
    loops = np.arange(N, dtype=np.int64)
    src = np.concatenate([src, loops])
    dst = np.concatenate([dst, loops])

    deg = np.bincount(dst, minlength=N)          # in-degree incl self-loop
    order = np.argsort(-deg, kind="stable")      # order[r] = node of rank r
    rank = np.empty(N, dtype=np.int64)
    rank[order] = np.arange(N)

    # global permuted-table position of each node
    ptab = (rank % NCORES) * NPC1 + rank // NCORES

    # K schedule: group g holds ranks [1024g, 1024(g+1)); max degree is at
    # the first rank of the stripe (degrees sorted descending)
    deg_sorted = deg[order]
    Kg = deg_sorted[np.arange(NG) * (NCORES * P)].astype(np.int64)
    offs = np.concatenate([[0], np.cumsum(Kg)])
    S = int(offs[-1])

    # ELL fill: edge (s -> d) goes to core/slot of d, column offs[g] + k
    # where k = index of the edge within d's in-edge list.
    eorder = np.argsort(dst, kind="stable")
    sdst = dst[eorder]
    ssrc = src[eorder]
    starts = np.cumsum(deg) - deg                # first edge index per dst
    ke = np.arange(sdst.shape[0]) - starts[sdst]

    rd = rank[sdst]
    c_e = rd % NCORES
    pos = rd // NCORES
    g_e = pos // P
    p_e = pos % P
    col = offs[g_e] + ke

    idx_arr = np.full((NCORES, P, S), DUMMY, dtype=np.int32)
    flat = (c_e * P + p_e) * S + col
    idx_arr.reshape(-1)[flat] = ptab[ssrc].astype(np.int32)

    # per-core transposed x slices (node-permuted, zero-padded)
    x = np.asarray(x, dtype=np.float32)
    xT = []
    for c in range(NCORES):
        xc = np.zeros((NPC, F_IN), dtype=np.float32)
        xc[:NREAL] = x[order[c::NCORES]]
        xT.append(np.ascontiguousarray(xc.T))

    # extended weights: fold alpha projections into the matmul
    W1 = np.asarray(W1, np.float32)
    W2 = np.asarray(W2, np.float32)
    w1e = np.concatenate(
        [W1, (W1 @ np.asarray(a_src1, np.float32))[:, None],
         (W1 @ np.asarray(a_dst1, np.float32))[:, None]], axis=1)  # [512, 66]
    w2e = np.zeros((HID, R2), dtype=np.float32)
    w2e[:, :NCLS] = W2
    w2e[:, NCLS] = W2 @ np.asarray(a_src2, np.float32)
    w2e[:, NCLS + 1] = W2 @ np.asarray(a_dst2, np.float32)

    b1r = np.ascontiguousarray(
        np.broadcast_to(np.asarray(b1, np.float32), (P, HID)))
    b2r = np.ascontiguousarray(
        np.broadcast_to(np.asarray(b2, np.float32), (P, NCLS)))

    dum1 = np.zeros((1, R1), dtype=np.float32)
    dum1[0, HID] = ALPHA_PAD
    dum2 = np.zeros((1, R2), dtype=np.float32)
    dum2[0, NCLS] = ALPHA_PAD

    in_maps = [
        {"xT": xT[c], "w1e": w1e, "w2e": w2e, "b1r": b1r, "b2r": b2r,
         "idx": idx_arr[c], "dum1": dum1, "dum2": dum2}
        for c in range(NCORES)
    ]
    return in_maps, Kg, offs, S, order


# ------------------------------------------------------------- bass program
def build_program(Kg, offs, S, compile_module=True):
    bass, mybir, tile = _import_bass()
    from concourse.masks import make_identity
    Alu = mybir.AluOpType
    Act = mybir.ActivationFunctionType
    X = mybir.AxisListType.X
    Kmax = int(max(Kg))

    nc = bacc.Bacc("TRN2", num_devices=NCORES)

    xT = nc.dram_tensor("xT", [F_IN, NPC], F32, kind="ExternalInput")
    w1e_d = nc.dram_tensor("w1e", [F_IN, R1], F32, kind="ExternalInput")
    w2e_d = nc.dram_tensor("w2e", [HID, R2], F32, kind="ExternalInput")
    b1r_d = nc.dram_tensor("b1r", [P, HID], F32, kind="ExternalInput")
    b2r_d = nc.dram_tensor("b2r", [P, NCLS], F32, kind="ExternalInput")
    idx_d = nc.dram_tensor("idx", [P, S], mybir.dt.int32, kind="ExternalInput")
    dum1_d = nc.dram_tensor("dum1", [1, R1], F32, kind="ExternalInput")
    dum2_d = nc.dram_tensor("dum2", [1, R2], F32, kind="ExternalInput")
    outp = nc.dram_tensor("out", [NPC, NCLS], F32, kind="ExternalOutput")
    if DEBUG_DUMPS:
        dbg_t1 = nc.dram_tensor("dbg_t1", [NT, R1], F32, kind="ExternalOutput")
        dbg_ad1 = nc.dram_tensor("dbg_ad1", [P, NG], F32,
                                 kind="ExternalOutput")
        dbg_g1 = nc.dram_tensor("dbg_g1", [P, int(Kg[0]) * R1], F32,
                                kind="ExternalOutput")
        dbg_w = nc.dram_tensor("dbg_w", [P, int(Kg[0])], F32,
                               kind="ExternalOutput")
        dbg_o1 = nc.dram_tensor("dbg_o1", [P, HID], F32,
                                kind="ExternalOutput")

    t1loc = nc.dram_tensor("t1loc", [NPC1, R1], F32)
    t2loc = nc.dram_tensor("t2loc", [NPC1, R2], F32)
    t1 = nc.dram_tensor("t1", [NT, R1], F32, addr_space="Shared")
    t2 = nc.dram_tensor("t2", [NT, R2], F32, addr_space="Shared")
    rg = [list(range(NCORES))]

    with tile.TileContext(nc) as tc:
        with (
            tc.tile_pool(name="const", bufs=1) as cpool,
            tc.tile_pool(name="xt", bufs=2) as xpool,
            tc.tile_pool(name="ps1", bufs=2, space="PSUM") as ps1,
            tc.tile_pool(name="pst", bufs=2, space="PSUM") as pst,
            tc.tile_pool(name="ps2", bufs=2, space="PSUM") as ps2,
            tc.tile_pool(name="work", bufs=2) as wp,
            tc.tile_pool(name="big", bufs=2) as bigp,
            tc.tile_pool(name="mbuf", bufs=1) as mp,
        ):
            # ---- resident constants
            w1t4 = cpool.tile([P, 4 * R1], F32, tag="w1t4")
            nc.sync.dma_start(
                out=w1t4[:].rearrange("p (c r) -> p c r", r=R1),
                in_=w1e_d[:].rearrange("(c p) r -> p c r", p=P))
            w1t = [w1t4[:, cc * R1:(cc + 1) * R1] for cc in range(4)]
            w2t = cpool.tile([HID, R2], F32, tag="w2t")
            nc.sync.dma_start(out=w2t[:], in_=w2e_d[:])
            b1t = cpool.tile([P, HID], F32, tag="b1t")
            nc.sync.dma_start(out=b1t[:], in_=b1r_d[:])
            b2t = cpool.tile([P, NCLS], F32, tag="b2t")
            nc.sync.dma_start(out=b2t[:], in_=b2r_d[:])
            idxt = cpool.tile([P, S], mybir.dt.int32, tag="idxt")
            nc.sync.dma_start(out=idxt[:], in_=idx_d[:])
            ident = cpool.tile([P, P], F32, tag="ident")
            make_identity(nc, ident[:])
            ad1 = cpool.tile([P, NG], F32, tag="ad1")
            ad2 = cpool.tile([P, NG], F32, tag="ad2")

            nc.sync.dma_start(out=t1loc[NPC:NPC + 1, :], in_=dum1_d[:])
            nc.sync.dma_start(out=t2loc[NPC:NPC + 1, :], in_=dum2_d[:])

            # ---- phase A: warm-up matmul observes w1t4's DMA tick so the
            # first real matmul carries only its xt wait (LW allows 1 wait)
            pwarm = pst.tile([R1, 1], F32, tag="pwarm")
            nc.tensor.matmul(pwarm[:], lhsT=w1t4[:, 0:R1],
                             rhs=w1t4[:, 0:1], start=True, stop=True)

            prev_copy = None
            for g in range(NG):
                xt4 = xpool.tile([P, 4 * P], F32, tag="xt4")
                nc.gpsimd.dma_start(
                    out=xt4[:].rearrange("p (c n) -> p c n", n=P),
                    in_=xT[:, g * P:(g + 1) * P].rearrange(
                        "(c p) n -> p c n", p=P))
                ph = ps1.tile([P, R1], F32, tag="ph1")
                mms = []
                for cc in range(4):
                    mms.append(nc.tensor.matmul(
                        ph[:], lhsT=xt4[:, cc * P:(cc + 1) * P],
                        rhs=w1t[cc], start=(cc == 0), stop=(cc == 3)))
                # hand the PSUM WAR tick to cc=3 (free wait slot): walrus
                # allows only ONE sync wait on a Matmult's LW stage, and
                # cc=0 already carries the xt DMA wait.
                if prev_copy is not None:
                    tile.add_dep_helper(
                        mms[3].ins, prev_copy.ins,
                        reason="psum WAR tick via cc3")
                tt = wp.tile([P, R1], F32, tag="tt1")
                prev_copy = nc.scalar.copy(out=tt[:], in_=ph[:])
                nc.sync.dma_start(
                    out=t1loc[g * P:(g + 1) * P, :], in_=tt[:])

            nc.gpsimd.collective_compute(
                "AllGather", mybir.AluOpType.bypass, replica_groups=rg,
                ins=[t1loc[:]], outs=[t1[:]])
            # alpha_dst columns for my own nodes, one strided DMA
            nc.gpsimd.dma_start(
                out=ad1[:],
                in_=t1loc[0:NPC, R1 - 1:R1].rearrange(
                    "(g p) o -> p (g o)", p=P))
            if DEBUG_DUMPS:
                nc.sync.dma_start(out=dbg_t1[:], in_=t1[:])
                nc.gpsimd.dma_start(out=dbg_ad1[:], in_=ad1[:])

            # ---- phase B: layer-1 edge aggregation, build local table2
            for g in range(NG):
                K = int(Kg[g])
                o = int(offs[g])
                G1 = bigp.tile([P, Kmax * R1], F32, tag="G1")
                for k in range(K):
                    nc.gpsimd.indirect_dma_start(
                        out=G1[:, k * R1:(k + 1) * R1],
                        out_offset=None, in_=t1[:],
                        in_offset=bass.IndirectOffsetOnAxis(
                            ap=idxt[:, o + k:o + k + 1], axis=0))
                g3 = G1[:, :K * R1].rearrange("p (k r) -> p k r", r=R1)

                e = wp.tile([P, Kmax], F32, tag="e1")
                nc.vector.tensor_scalar_add(
                    e[:, :K], g3[:, :, HID:HID + 1], ad1[:, g:g + 1])
                nc.vector.scalar_tensor_tensor(
                    out=e[:, :K], in0=e[:, :K], scalar=NEG_SLOPE,
                    in1=e[:, :K], op0=Alu.mult, op1=Alu.max)
                w = wp.tile([P, Kmax], F32, tag="w1")
                nc.scalar.activation(w[:, :K], e[:, :K], Act.Exp)
                den = wp.tile([P, 1], F32, tag="den1")
                nc.vector.tensor_reduce(
                    out=den[:], in_=w[:, :K], axis=X, op=Alu.add)
                nc.vector.tensor_scalar_add(den[:], den[:], 1e-16)
                rc = wp.tile([P, 1], F32, tag="rc1")
                nc.vector.reciprocal(rc[:], den[:])

                M = mp.tile([P, Kmax * HID], F32, tag="M1")
                m3 = M[:, :K * HID].rearrange("p (k r) -> p k r", r=HID)
                nc.vector.tensor_tensor(
                    out=m3, in0=g3[:, :, :HID],
                    in1=w[:, :K].unsqueeze(2).to_broadcast([P, K, HID]),
                    op=Alu.mult)
                red = wp.tile([P, HID], F32, tag="red1")
                nc.vector.tensor_reduce(
                    out=red[:],
                    in_=M[:, :K * HID].rearrange("p (k r) -> p r k", r=HID),
                    axis=X, op=Alu.add)
                o1 = wp.tile([P, HID], F32, tag="o1")
                nc.vector.scalar_tensor_tensor(
                    out=o1[:], in0=red[:], scalar=rc[:], in1=b1t[:],
                    op0=Alu.mult, op1=Alu.add)
                if DEBUG_DUMPS and g == 0:
                    nc.sync.dma_start(out=dbg_g1[:], in_=G1[:, :K * R1])
                    nc.sync.dma_start(out=dbg_w[:], in_=w[:, :K])
                    nc.sync.dma_start(out=dbg_o1[:], in_=o1[:])

                # ELU(x) = relu(x) + exp(min(x,0)) - 1
                tmin = wp.tile([P, HID], F32, tag="tmin")
                nc.vector.tensor_scalar_min(tmin[:], o1[:], 0.0)
                texp = wp.tile([P, HID], F32, tag="texp")
                nc.scalar.activation(texp[:], tmin[:], Act.Exp)
                trelu = wp.tile([P, HID], F32, tag="trelu")
                nc.vector.tensor_scalar_max(trelu[:], o1[:], 0.0)
                h1e = wp.tile([P, HID], F32, tag="h1e")
                nc.vector.scalar_tensor_tensor(
                    out=h1e[:], in0=texp[:], scalar=-1.0, in1=trelu[:],
                    op0=Alu.add, op1=Alu.add)

                # h2 = h1e @ W2e  (transpose h1e, contract over HID)
                pt = pst.tile([HID, P], F32, tag="pt")
                nc.tensor.transpose(pt[:], h1e[:], ident[:])
                h1eT = wp.tile([HID, P], F32, tag="h1eT")
                nc.vector.tensor_copy(h1eT[:], pt[:])
                ph2 = ps2.tile([P, R2], F32, tag="ph2")
                nc.tensor.matmul(
                    ph2[:], lhsT=h1eT[:], rhs=w2t[:], start=True, stop=True)
                t2t = wp.tile([P, R2], F32, tag="t2t")
                nc.vector.tensor_copy(t2t[:], ph2[:])
                nc.sync.dma_start(
                    out=t2loc[g * P:(g + 1) * P, :], in_=t2t[:])

            nc.gpsimd.collective_compute(
                "AllGather", mybir.AluOpType.bypass, replica_groups=rg,
                ins=[t2loc[:]], outs=[t2[:]])
            nc.gpsimd.dma_start(
                out=ad2[:],
                in_=t2loc[0:NPC, NCLS + 1:NCLS + 2].rearrange(
                    "(g p) o -> p (g o)", p=P))

            # ---- phase C: layer-2 edge aggregation + log_softmax
            for g in range(NG):
                K = int(Kg[g])
                o = int(offs[g])
                G2 = bigp.tile([P, Kmax * R2], F32, tag="G2")
                for k in range(K):
                    nc.gpsimd.indirect_dma_start(
                        out=G2[:, k * R2:(k + 1) * R2],
                        out_offset=None, in_=t2[:],
                        in_offset=bass.IndirectOffsetOnAxis(
                            ap=idxt[:, o + k:o + k + 1], axis=0))
                q3 = G2[:, :K * R2].rearrange("p (k r) -> p k r", r=R2)

                e = wp.tile([P, Kmax], F32, tag="e2")
                nc.vector.tensor_scalar_add(
                    e[:, :K], q3[:, :, NCLS:NCLS + 1], ad2[:, g:g + 1])
                nc.vector.scalar_tensor_tensor(
                    out=e[:, :K], in0=e[:, :K], scalar=NEG_SLOPE,
                    in1=e[:, :K], op0=Alu.mult, op1=Alu.max)
                w = wp.tile([P, Kmax], F32, tag="w2")
                nc.scalar.activation(w[:, :K], e[:, :K], Act.Exp)
                den = wp.tile([P, 1], F32, tag="den2")
                nc.vector.tensor_reduce(
                    out=den[:], in_=w[:, :K], axis=X, op=Alu.add)
                nc.vector.tensor_scalar_add(den[:], den[:], 1e-16)
                rc = wp.tile([P, 1], F32, tag="rc2")
                nc.vector.reciprocal(rc[:], den[:])

                M = mp.tile([P, Kmax * NCLS], F32, tag="M2")
                m3 = M[:, :K * NCLS].rearrange("p (k r) -> p k r", r=NCLS)
                nc.vector.tensor_tensor(
                    out=m3, in0=q3[:, :, :NCLS],
                    in1=w[:, :K].unsqueeze(2).to_broadcast([P, K, NCLS]),
                    op=Alu.mult)
                red = wp.tile([P, NCLS], F32, tag="red2")
                nc.vector.tensor_reduce(
                    out=red[:],
                    in_=M[:, :K * NCLS].rearrange("p (k r) -> p r k", r=NCLS),
                    axis=X, op=Alu.add)
                logits = wp.tile([P, NCLS], F32, tag="logits")
                nc.vector.scalar_tensor_tensor(
                    out=logits[:], in0=red[:], scalar=rc[:], in1=b2t[:],
                    op0=Alu.mult, op1=Alu.add)

                # log_softmax
                mx = wp.tile([P, 1], F32, tag="mx")
                nc.vector.tensor_reduce(
                    out=mx[:], in_=logits[:], axis=X, op=Alu.max)
                sh = wp.tile([P, NCLS], F32, tag="sh")
                nc.vector.tensor_scalar_sub(sh[:], logits[:], mx[:])
                es = wp.tile([P, NCLS], F32, tag="es")
                nc.scalar.activation(es[:], sh[:], Act.Exp)
                ssum = wp.tile([P, 1], F32, tag="ssum")
                nc.vector.tensor_reduce(
                    out=ssum[:], in_=es[:], axis=X, op=Alu.add)
                lg = wp.tile([P, 1], F32, tag="lg")
                nc.scalar.activation(lg[:], ssum[:], Act.Ln)
                fin = wp.tile([P, NCLS], F32, tag="fin")
                nc.vector.tensor_scalar_sub(fin[:], sh[:], lg[:])
                nc.sync.dma_start(
                    out=outp[g * P:(g + 1) * P, :], in_=fin[:])

    if compile_module:
        nc.compile()
    return nc


# ------------------------------------------------------------------- runner
_CACHE = {}


def kernel_with_results(inputs, trace=False):
    in_maps, Kg, offs, S, order = preprocess(**inputs)
    key = (tuple(Kg.tolist()), S)
    if key not in _CACHE:
        _CACHE[key] = build_program(Kg, offs, S)
    nc = _CACHE[key]
    from concourse.bass_utils import run_bass_kernel_spmd
    res = run_bass_kernel_spmd(nc, in_maps, list(range(NCORES)), trace=trace)
    out = np.empty((N, NCLS), dtype=np.float32)
    for c in range(NCORES):
        out[order[c::NCORES]] = res.results[c]["out"][:NREAL]
    return out, res


def kernel(**inputs):
    out, _ = kernel_with_results(inputs, trace=False)
    return out


# revision 18
# speedup vs baseline: 1.0692x; 1.0692x over previous
"""Two-layer GAT (GATConv heads=1, PyG-style) on 8 Trainium2 NeuronCores.

Strategy (matches sharding_hint): nodes are degree-sorted and dealt
round-robin to the 8 cores (so every core sees the same degree profile,
letting one SPMD program with a fixed per-group ELL K-schedule serve all
cores). Edges are partitioned by destination node. Each core computes the
feature transform for its own node slice, the per-node tables
[h | alpha_src (| alpha_dst)] are AllGathered so each core holds the full
table in local HBM, and edge aggregation is done with multi-index indirect
DMA gathers (ELL layout: node = SBUF partition, edge slot = free dim)
followed by a segment softmax + weighted reduction on the vector/scalar
engines. Layer 2 repeats the pattern with a second AllGather.
"""
import sys

sys.path.insert(0, "/opt/trn_rl_repo")

import numpy as np

# ---------------------------------------------------------------- constants
N = 50000        # nodes
F_IN = 512       # input features
HID = 64         # layer-1 out features
NCLS = 40        # classes
NEG_SLOPE = 0.2

NCORES = 8
P = 128                       # SBUF partitions
NG = 49                       # node groups per core
NPC = NG * P                  # node slots per core (6272; 6250 real)
NREAL = N // NCORES           # real nodes per core (6250)
DEBUG_DUMPS = False           # add intermediate ExternalOutputs (debug only)
NPC1 = NPC + 1                # slice rows incl per-core dummy row
NT = NCORES * NPC1            # global permuted table rows (50184)
DUMMY = NPC                   # dummy row index = core-0's dummy slot
ALPHA_PAD = -60.0

R1 = HID + 2                  # table1 row: [h(64) | a_src | a_dst] = 66
R2 = 44                       # table2 row: [h2(40) | a_src | a_dst | pad] = 44

F32 = None  # filled after imports


def _import_bass():
    global bass, bacc, mybir, tile, F32
    import concourse.bass as bass
    import concourse.bacc as bacc
    import concourse.mybir as mybir
    import concourse.tile as tile
    F32 = mybir.dt.float32
    return bass, mybir, tile


# ---------------------------------------------------------------- host prep
def preprocess(x, edge_index, W1, a_src1, a_dst1, b1, W2, a_src2, a_dst2, b2):
    """Degree-sort nodes, deal round-robin to cores, build ELL edge arrays."""
    src = np.asarray(edge_index[0], dtype=np.int64)
    dst = np.asarray(edge_index[1], dtype=np.int64)
    loops = np.arange(N, dtype=np.int64)
    src = np.concatenate([src, loops])
    dst = np.concatenate([dst, loops])

    deg = np.bincount(dst, minlength=N)          # in-degree incl self-loop
    order = np.argsort(-deg, kind="stable")      # order[r] = node of rank r
    rank = np.empty(N, dtype=np.int64)
    rank[order] = np.arange(N)

    # global permuted-table position of each node
    ptab = (rank % NCORES) * NPC1 + rank // NCORES

    # K schedule: group g holds ranks [1024g, 1024(g+1)); max degree is at
    # the first rank of the stripe (degrees sorted descending)
    deg_sorted = deg[order]
    Kg = deg_sorted[np.arange(NG) * (NCORES * P)].astype(np.int64)
    offs = np.concatenate([[0], np.cumsum(Kg)])
    S = int(offs[-1])

    # ELL fill: edge (s -> d) goes to core/slot of d, column offs[g] + k
    # where k = index of the edge within d's in-edge list.
    eorder = np.argsort(dst, kind="stable")
    sdst = dst[eorder]
    ssrc = src[eorder]
    starts = np.cumsum(deg) - deg                # first edge index per dst
    ke = np.arange(sdst.shape[0]) - starts[sdst]

    rd = rank[sdst]
    c_e = rd % NCORES
    pos = rd // NCORES
    g_e = pos // P
    p_e = pos % P
    col = offs[g_e] + ke

    idx_arr = np.full((NCORES, P, S), DUMMY, dtype=np.int32)
    flat = (c_e * P + p_e) * S + col
    idx_arr.reshape(-1)[flat] = ptab[ssrc].astype(np.int32)

    # per-core transposed x slices (node-permuted, zero-padded)
    x = np.asarray(x, dtype=np.float32)
    xT = []
    for c in range(NCORES):
        xc = np.zeros((NPC, F_IN), dtype=np.float32)
        xc[:NREAL] = x[order[c::NCORES]]
        xT.append(np.ascontiguousarray(xc.T))

    # extended weights: fold alpha projections into the matmul
    W1 = np.asarray(W1, np.float32)
    W2 = np.asarray(W2, np.float32)
    w1e = np.concatenate(
        [W1, (W1 @ np.asarray(a_src1, np.float32))[:, None],
         (W1 @ np.asarray(a_dst1, np.float32))[:, None]], axis=1)  # [512, 66]
    w2e = np.zeros((HID, R2), dtype=np.float32)
    w2e[:, :NCLS] = W2
    w2e[:, NCLS] = W2 @ np.asarray(a_src2, np.float32)
    w2e[:, NCLS + 1] = W2 @ np.asarray(a_dst2, np.float32)

    b1r = np.ascontiguousarray(
        np.broadcast_to(np.asarray(b1, np.float32), (P, HID)))
    b2r = np.ascontiguousarray(
        np.broadcast_to(np.asarray(b2, np.float32), (P, NCLS)))

    dum1 = np.zeros((1, R1), dtype=np.float32)
    dum1[0, HID] = ALPHA_PAD
    dum2 = np.zeros((1, R2), dtype=np.float32)
    dum2[0, NCLS] = ALPHA_PAD

    in_maps = [
        {"xT": xT[c], "w1e": w1e, "w2e": w2e, "b1r": b1r, "b2r": b2r,
         "idx": idx_arr[c], "dum1": dum1, "dum2": dum2}
        for c in range(NCORES)
    ]
    return in_maps, Kg, offs, S, order


# ------------------------------------------------------------- bass program
def build_program(Kg, offs, S, compile_module=True):
    bass, mybir, tile = _import_bass()
    from concourse.masks import make_identity
    Alu = mybir.AluOpType
    Act = mybir.ActivationFunctionType
    X = mybir.AxisListType.X
    Kmax = int(max(Kg))

    nc = bacc.Bacc("TRN2", num_devices=NCORES)

    xT = nc.dram_tensor("xT", [F_IN, NPC], F32, kind="ExternalInput")
    w1e_d = nc.dram_tensor("w1e", [F_IN, R1], F32, kind="ExternalInput")
    w2e_d = nc.dram_tensor("w2e", [HID, R2], F32, kind="ExternalInput")
    b1r_d = nc.dram_tensor("b1r", [P, HID], F32, kind="ExternalInput")
    b2r_d = nc.dram_tensor("b2r", [P, NCLS], F32, kind="ExternalInput")
    idx_d = nc.dram_tensor("idx", [P, S], mybir.dt.int32, kind="ExternalInput")
    dum1_d = nc.dram_tensor("dum1", [1, R1], F32, kind="ExternalInput")
    dum2_d = nc.dram_tensor("dum2", [1, R2], F32, kind="ExternalInput")
    outp = nc.dram_tensor("out", [NPC, NCLS], F32, kind="ExternalOutput")
    if DEBUG_DUMPS:
        dbg_t1 = nc.dram_tensor("dbg_t1", [NT, R1], F32, kind="ExternalOutput")
        dbg_ad1 = nc.dram_tensor("dbg_ad1", [P, NG], F32,
                                 kind="ExternalOutput")
        dbg_g1 = nc.dram_tensor("dbg_g1", [P, int(Kg[0]) * R1], F32,
                                kind="ExternalOutput")
        dbg_w = nc.dram_tensor("dbg_w", [P, int(Kg[0])], F32,
                               kind="ExternalOutput")
        dbg_o1 = nc.dram_tensor("dbg_o1", [P, HID], F32,
                                kind="ExternalOutput")

    t1loc = nc.dram_tensor("t1loc", [NPC1, R1], F32)
    t2loc = nc.dram_tensor("t2loc", [NPC1, R2], F32)
    t1 = nc.dram_tensor("t1", [NT, R1], F32, addr_space="Shared")
    t2 = nc.dram_tensor("t2", [NT, R2], F32, addr_space="Shared")
    rg = [list(range(NCORES))]

    with tile.TileContext(nc) as tc:
        with (
            tc.tile_pool(name="const", bufs=1) as cpool,
            tc.tile_pool(name="xt", bufs=2) as xpool,
            tc.tile_pool(name="ps1", bufs=2, space="PSUM") as ps1,
            tc.tile_pool(name="pst", bufs=2, space="PSUM") as pst,
            tc.tile_pool(name="ps2", bufs=2, space="PSUM") as ps2,
            tc.tile_pool(name="work", bufs=2) as wp,
            tc.tile_pool(name="big", bufs=3) as bigp,
            tc.tile_pool(name="mbuf", bufs=1) as mp,
        ):
            # ---- resident constants
            w1t4 = cpool.tile([P, 4 * R1], F32, tag="w1t4")
            nc.sync.dma_start(
                out=w1t4[:].rearrange("p (c r) -> p c r", r=R1),
                in_=w1e_d[:].rearrange("(c p) r -> p c r", p=P))
            w1t = [w1t4[:, cc * R1:(cc + 1) * R1] for cc in range(4)]
            w2t = cpool.tile([HID, R2], F32, tag="w2t")
            nc.sync.dma_start(out=w2t[:], in_=w2e_d[:])
            b1t = cpool.tile([P, HID], F32, tag="b1t")
            nc.sync.dma_start(out=b1t[:], in_=b1r_d[:])
            b2t = cpool.tile([P, NCLS], F32, tag="b2t")
            nc.sync.dma_start(out=b2t[:], in_=b2r_d[:])
            idxt = cpool.tile([P, S], mybir.dt.int32, tag="idxt")
            nc.sync.dma_start(out=idxt[:], in_=idx_d[:])
            ident = cpool.tile([P, P], F32, tag="ident")
            make_identity(nc, ident[:])
            ad1 = cpool.tile([P, NG], F32, tag="ad1")
            ad2 = cpool.tile([P, NG], F32, tag="ad2")

            nc.sync.dma_start(out=t1loc[NPC:NPC + 1, :], in_=dum1_d[:])
            nc.sync.dma_start(out=t2loc[NPC:NPC + 1, :], in_=dum2_d[:])

            # ---- phase A: warm-up matmul observes w1t4's DMA tick so the
            # first real matmul carries only its xt wait (LW allows 1 wait)
            pwarm = pst.tile([R1, 1], F32, tag="pwarm")
            nc.tensor.matmul(pwarm[:], lhsT=w1t4[:, 0:R1],
                             rhs=w1t4[:, 0:1], start=True, stop=True)

            prev_copy = None
            for g in range(NG):
                xt4 = xpool.tile([P, 4 * P], F32, tag="xt4")
                nc.sync.dma_start(
                    out=xt4[:].rearrange("p (c n) -> p c n", n=P),
                    in_=xT[:, g * P:(g + 1) * P].rearrange(
                        "(c p) n -> p c n", p=P))
                ph = ps1.tile([P, R1], F32, tag="ph1")
                mms = []
                for cc in range(4):
                    mms.append(nc.tensor.matmul(
                        ph[:], lhsT=xt4[:, cc * P:(cc + 1) * P],
                        rhs=w1t[cc], start=(cc == 0), stop=(cc == 3)))
                # hand the PSUM WAR tick to cc=3 (free wait slot): walrus
                # allows only ONE sync wait on a Matmult's LW stage, and
                # cc=0 already carries the xt DMA wait.
                if prev_copy is not None:
                    tile.add_dep_helper(
                        mms[3].ins, prev_copy.ins,
                        reason="psum WAR tick via cc3")
                tt = wp.tile([P, R1], F32, tag="tt1")
                prev_copy = nc.scalar.copy(out=tt[:], in_=ph[:])
                nc.sync.dma_start(
                    out=t1loc[g * P:(g + 1) * P, :], in_=tt[:])

            nc.gpsimd.collective_compute(
                "AllGather", mybir.AluOpType.bypass, replica_groups=rg,
                ins=[t1loc[:]], outs=[t1[:]])
            # alpha_dst columns for my own nodes, one strided DMA
            nc.scalar.dma_start(
                out=ad1[:],
                in_=t1loc[0:NPC, R1 - 1:R1].rearrange(
                    "(g p) o -> p (g o)", p=P))
            if DEBUG_DUMPS:
                nc.sync.dma_start(out=dbg_t1[:], in_=t1[:])
                nc.gpsimd.dma_start(out=dbg_ad1[:], in_=ad1[:])

            # ---- phase B: layer-1 edge aggregation, build local table2
            for g in range(NG):
                K = int(Kg[g])
                o = int(offs[g])
                G1 = bigp.tile([P, Kmax * R1], F32, tag="G1")
                for k in range(K):
                    nc.gpsimd.indirect_dma_start(
                        out=G1[:, k * R1:(k + 1) * R1],
                        out_offset=None, in_=t1[:],
                        in_offset=bass.IndirectOffsetOnAxis(
                            ap=idxt[:, o + k:o + k + 1], axis=0))
                g3 = G1[:, :K * R1].rearrange("p (k r) -> p k r", r=R1)

                e = wp.tile([P, Kmax], F32, tag="e1")
                nc.vector.tensor_scalar_add(
                    e[:, :K], g3[:, :, HID:HID + 1], ad1[:, g:g + 1])
                nc.vector.scalar_tensor_tensor(
                    out=e[:, :K], in0=e[:, :K], scalar=NEG_SLOPE,
                    in1=e[:, :K], op0=Alu.mult, op1=Alu.max)
                w = wp.tile([P, Kmax], F32, tag="w1")
                nc.scalar.activation(w[:, :K], e[:, :K], Act.Exp)
                den = wp.tile([P, 1], F32, tag="den1")
                nc.vector.tensor_reduce(
                    out=den[:], in_=w[:, :K], axis=X, op=Alu.add)
                nc.vector.tensor_scalar_add(den[:], den[:], 1e-16)
                rc = wp.tile([P, 1], F32, tag="rc1")
                nc.vector.reciprocal(rc[:], den[:])

                M = mp.tile([P, Kmax * HID], F32, tag="M1")
                m3 = M[:, :K * HID].rearrange("p (k r) -> p k r", r=HID)
                nc.vector.tensor_tensor(
                    out=m3, in0=g3[:, :, :HID],
                    in1=w[:, :K].unsqueeze(2).to_broadcast([P, K, HID]),
                    op=Alu.mult)
                red = wp.tile([P, HID], F32, tag="red1")
                nc.vector.tensor_reduce(
                    out=red[:],
                    in_=M[:, :K * HID].rearrange("p (k r) -> p r k", r=HID),
                    axis=X, op=Alu.add)
                o1 = wp.tile([P, HID], F32, tag="o1")
                nc.vector.scalar_tensor_tensor(
                    out=o1[:], in0=red[:], scalar=rc[:], in1=b1t[:],
                    op0=Alu.mult, op1=Alu.add)
                if DEBUG_DUMPS and g == 0:
                    nc.sync.dma_start(out=dbg_g1[:], in_=G1[:, :K * R1])
                    nc.sync.dma_start(out=dbg_w[:], in_=w[:, :K])
                    nc.sync.dma_start(out=dbg_o1[:], in_=o1[:])

                # ELU(x) = relu(x) + exp(min(x,0)) - 1
                tmin = wp.tile([P, HID], F32, tag="tmin")
                nc.vector.tensor_scalar_min(tmin[:], o1[:], 0.0)
                texp = wp.tile([P, HID], F32, tag="texp")
                nc.scalar.activation(texp[:], tmin[:], Act.Exp)
                trelu = wp.tile([P, HID], F32, tag="trelu")
                nc.vector.tensor_scalar_max(trelu[:], o1[:], 0.0)
                h1e = wp.tile([P, HID], F32, tag="h1e")
                nc.vector.scalar_tensor_tensor(
                    out=h1e[:], in0=texp[:], scalar=-1.0, in1=trelu[:],
                    op0=Alu.add, op1=Alu.add)

                # h2 = h1e @ W2e  (transpose h1e, contract over HID)
                pt = pst.tile([HID, P], F32, tag="pt")
                nc.tensor.transpose(pt[:], h1e[:], ident[:])
                h1eT = wp.tile([HID, P], F32, tag="h1eT")
                nc.vector.tensor_copy(h1eT[:], pt[:])
                ph2 = ps2.tile([P, R2], F32, tag="ph2")
                nc.tensor.matmul(
                    ph2[:], lhsT=h1eT[:], rhs=w2t[:], start=True, stop=True)
                t2t = wp.tile([P, R2], F32, tag="t2t")
                nc.vector.tensor_copy(t2t[:], ph2[:])
                nc.sync.dma_start(
                    out=t2loc[g * P:(g + 1) * P, :], in_=t2t[:])

            nc.gpsimd.collective_compute(
                "AllGather", mybir.AluOpType.bypass, replica_groups=rg,
                ins=[t2loc[:]], outs=[t2[:]])
            nc.scalar.dma_start(
                out=ad2[:],
                in_=t2loc[0:NPC, NCLS + 1:NCLS + 2].rearrange(
                    "(g p) o -> p (g o)", p=P))

            # ---- phase C: layer-2 edge aggregation + log_softmax
            for g in range(NG):
                K = int(Kg[g])
                o = int(offs[g])
                G2 = bigp.tile([P, Kmax * R2], F32, tag="G2")
                for k in range(K):
                    nc.gpsimd.indirect_dma_start(
                        out=G2[:, k * R2:(k + 1) * R2],
                        out_offset=None, in_=t2[:],
                        in_offset=bass.IndirectOffsetOnAxis(
                            ap=idxt[:, o + k:o + k + 1], axis=0))
                q3 = G2[:, :K * R2].rearrange("p (k r) -> p k r", r=R2)

                e = wp.tile([P, Kmax], F32, tag="e2")
                nc.vector.tensor_scalar_add(
                    e[:, :K], q3[:, :, NCLS:NCLS + 1], ad2[:, g:g + 1])
                nc.vector.scalar_tensor_tensor(
                    out=e[:, :K], in0=e[:, :K], scalar=NEG_SLOPE,
                    in1=e[:, :K], op0=Alu.mult, op1=Alu.max)
                w = wp.tile([P, Kmax], F32, tag="w2")
                nc.scalar.activation(w[:, :K], e[:, :K], Act.Exp)
                den = wp.tile([P, 1], F32, tag="den2")
                nc.vector.tensor_reduce(
                    out=den[:], in_=w[:, :K], axis=X, op=Alu.add)
                nc.vector.tensor_scalar_add(den[:], den[:], 1e-16)
                rc = wp.tile([P, 1], F32, tag="rc2")
                nc.vector.reciprocal(rc[:], den[:])

                M = mp.tile([P, Kmax * NCLS], F32, tag="M2")
                m3 = M[:, :K * NCLS].rearrange("p (k r) -> p k r", r=NCLS)
                nc.vector.tensor_tensor(
                    out=m3, in0=q3[:, :, :NCLS],
                    in1=w[:, :K].unsqueeze(2).to_broadcast([P, K, NCLS]),
                    op=Alu.mult)
                red = wp.tile([P, NCLS], F32, tag="red2")
                nc.vector.tensor_reduce(
                    out=red[:],
                    in_=M[:, :K * NCLS].rearrange("p (k r) -> p r k", r=NCLS),
                    axis=X, op=Alu.add)
                logits = wp.tile([P, NCLS], F32, tag="logits")
                nc.vector.scalar_tensor_tensor(
                    out=logits[:], in0=red[:], scalar=rc[:], in1=b2t[:],
                    op0=Alu.mult, op1=Alu.add)

                # log_softmax
                mx = wp.tile([P, 1], F32, tag="mx")
                nc.vector.tensor_reduce(
                    out=mx[:], in_=logits[:], axis=X, op=Alu.max)
                sh = wp.tile([P, NCLS], F32, tag="sh")
                nc.vector.tensor_scalar_sub(sh[:], logits[:], mx[:])
                es = wp.tile([P, NCLS], F32, tag="es")
                nc.scalar.activation(es[:], sh[:], Act.Exp)
                ssum = wp.tile([P, 1], F32, tag="ssum")
                nc.vector.tensor_reduce(
                    out=ssum[:], in_=es[:], axis=X, op=Alu.add)
                lg = wp.tile([P, 1], F32, tag="lg")
                nc.scalar.activation(lg[:], ssum[:], Act.Ln)
                fin = wp.tile([P, NCLS], F32, tag="fin")
                nc.vector.tensor_scalar_sub(fin[:], sh[:], lg[:])
                nc.sync.dma_start(
                    out=outp[g * P:(g + 1) * P, :], in_=fin[:])

    if compile_module:
        nc.compile()
    return nc


# ------------------------------------------------------------------- runner
_CACHE = {}


def kernel_with_results(inputs, trace=False):
    in_maps, Kg, offs, S, order = preprocess(**inputs)
    key = (tuple(Kg.tolist()), S)
    if key not in _CACHE:
        _CACHE[key] = build_program(Kg, offs, S)
    nc = _CACHE[key]
    from concourse.bass_utils import run_bass_kernel_spmd
    res = run_bass_kernel_spmd(nc, in_maps, list(range(NCORES)), trace=trace)
    out = np.empty((N, NCLS), dtype=np.float32)
    for c in range(NCORES):
        out[order[c::NCORES]] = res.results[c]["out"][:NREAL]
    return out, res


def kernel(**inputs):
    out, _ = kernel_with_results(inputs, trace=False)
    return out


# revision 20
# speedup vs baseline: 1.3379x; 1.2514x over previous
"""Two-layer GAT (GATConv heads=1, PyG-style) on 8 Trainium2 NeuronCores.

Strategy (matches sharding_hint): nodes are degree-sorted and dealt
round-robin to the 8 cores (so every core sees the same degree profile,
letting one SPMD program with a fixed per-group ELL K-schedule serve all
cores). Edges are partitioned by destination node. Each core computes the
feature transform for its own node slice, the per-node tables
[h | alpha_src (| alpha_dst)] are AllGathered so each core holds the full
table in local HBM, and edge aggregation is done with multi-index indirect
DMA gathers (ELL layout: node = SBUF partition, edge slot = free dim)
followed by a segment softmax + weighted reduction on the vector/scalar
engines. Layer 2 repeats the pattern with a second AllGather.
"""
import sys

sys.path.insert(0, "/opt/trn_rl_repo")

import numpy as np

# ---------------------------------------------------------------- constants
N = 50000        # nodes
F_IN = 512       # input features
HID = 64         # layer-1 out features
NCLS = 40        # classes
NEG_SLOPE = 0.2

NCORES = 8
P = 128                       # SBUF partitions
NG = 49                       # node groups per core
NPC = NG * P                  # node slots per core (6272; 6250 real)
NREAL = N // NCORES           # real nodes per core (6250)
DEBUG_DUMPS = False           # add intermediate ExternalOutputs (debug only)
NPC1 = NPC + 1                # slice rows incl per-core dummy row
NT = NCORES * NPC1            # global permuted table rows (50184)
DUMMY = NPC                   # dummy row index = core-0's dummy slot
ALPHA_PAD = -60.0

R1 = HID + 2                  # table1 row: [h(64) | a_src | a_dst] = 66
R2 = 44                       # table2 row: [h2(40) | a_src | a_dst | pad] = 44

F32 = None  # filled after imports


def _import_bass():
    global bass, bacc, mybir, tile, F32
    import concourse.bass as bass
    import concourse.bacc as bacc
    import concourse.mybir as mybir
    import concourse.tile as tile
    F32 = mybir.dt.float32
    return bass, mybir, tile


# ---------------------------------------------------------------- host prep
def preprocess(x, edge_index, W1, a_src1, a_dst1, b1, W2, a_src2, a_dst2, b2):
    """Degree-sort nodes, deal round-robin to cores, build ELL edge arrays."""
    src = np.asarray(edge_index[0], dtype=np.int64)
    dst = np.asarray(edge_index[1], dtype=np.int64)
    loops = np.arange(N, dtype=np.int64)
    src = np.concatenate([loops, src])
    dst = np.concatenate([loops, dst])

    deg = np.bincount(dst, minlength=N)          # in-degree incl self-loop
    order = np.argsort(-deg, kind="stable")      # order[r] = node of rank r
    rank = np.empty(N, dtype=np.int64)
    rank[order] = np.arange(N)

    # global permuted-table position of each node
    ptab = (rank % NCORES) * NPC1 + rank // NCORES

    # K schedule: group g holds ranks [1024g, 1024(g+1)); max degree is at
    # the first rank of the stripe (degrees sorted descending)
    deg_sorted = deg[order]
    Kg = deg_sorted[np.arange(NG) * (NCORES * P)].astype(np.int64)
    offs = np.concatenate([[0], np.cumsum(Kg)])
    S = int(offs[-1])

    # ELL fill: edge (s -> d) goes to core/slot of d, column offs[g] + k
    # where k = index of the edge within d's in-edge list.
    eorder = np.argsort(dst, kind="stable")
    sdst = dst[eorder]
    ssrc = src[eorder]
    starts = np.cumsum(deg) - deg                # first edge index per dst
    ke = np.arange(sdst.shape[0]) - starts[sdst]

    rd = rank[sdst]
    c_e = rd % NCORES
    pos = rd // NCORES
    g_e = pos // P
    p_e = pos % P
    col = offs[g_e] + ke

    idx_arr = np.full((NCORES, P, S), DUMMY, dtype=np.int32)
    flat = (c_e * P + p_e) * S + col
    idx_arr.reshape(-1)[flat] = ptab[ssrc].astype(np.int32)

    # per-core transposed x slices (node-permuted, zero-padded)
    x = np.asarray(x, dtype=np.float32)
    xT = []
    for c in range(NCORES):
        xc = np.zeros((NPC, F_IN), dtype=np.float32)
        xc[:NREAL] = x[order[c::NCORES]]
        xT.append(np.ascontiguousarray(xc.T))

    # extended weights: fold alpha projections into the matmul
    W1 = np.asarray(W1, np.float32)
    W2 = np.asarray(W2, np.float32)
    w1e = np.concatenate(
        [W1, (W1 @ np.asarray(a_src1, np.float32))[:, None],
         (W1 @ np.asarray(a_dst1, np.float32))[:, None]], axis=1)  # [512, 66]
    w2e = np.zeros((HID, R2), dtype=np.float32)
    w2e[:, :NCLS] = W2
    w2e[:, NCLS] = W2 @ np.asarray(a_src2, np.float32)
    w2e[:, NCLS + 1] = W2 @ np.asarray(a_dst2, np.float32)

    b1r = np.ascontiguousarray(
        np.broadcast_to(np.asarray(b1, np.float32), (P, HID)))
    b2r = np.ascontiguousarray(
        np.broadcast_to(np.asarray(b2, np.float32), (P, NCLS)))

    dum1 = np.zeros((1, R1), dtype=np.float32)
    dum1[0, HID] = ALPHA_PAD
    dum2 = np.zeros((1, R2), dtype=np.float32)
    dum2[0, NCLS] = ALPHA_PAD

    in_maps = [
        {"xT": xT[c], "w1e": w1e, "w2e": w2e, "b1r": b1r, "b2r": b2r,
         "idx": idx_arr[c], "dum1": dum1, "dum2": dum2}
        for c in range(NCORES)
    ]
    return in_maps, Kg, offs, S, order


# ------------------------------------------------------------- bass program
def build_program(Kg, offs, S, compile_module=True):
    bass, mybir, tile = _import_bass()
    from concourse.masks import make_identity
    Alu = mybir.AluOpType
    Act = mybir.ActivationFunctionType
    X = mybir.AxisListType.X
    Kmax = int(max(Kg))

    nc = bacc.Bacc("TRN2", num_devices=NCORES)

    xT = nc.dram_tensor("xT", [F_IN, NPC], F32, kind="ExternalInput")
    w1e_d = nc.dram_tensor("w1e", [F_IN, R1], F32, kind="ExternalInput")
    w2e_d = nc.dram_tensor("w2e", [HID, R2], F32, kind="ExternalInput")
    b1r_d = nc.dram_tensor("b1r", [P, HID], F32, kind="ExternalInput")
    b2r_d = nc.dram_tensor("b2r", [P, NCLS], F32, kind="ExternalInput")
    idx_d = nc.dram_tensor("idx", [P, S], mybir.dt.int32, kind="ExternalInput")
    dum1_d = nc.dram_tensor("dum1", [1, R1], F32, kind="ExternalInput")
    dum2_d = nc.dram_tensor("dum2", [1, R2], F32, kind="ExternalInput")
    outp = nc.dram_tensor("out", [NPC, NCLS], F32, kind="ExternalOutput")
    if DEBUG_DUMPS:
        dbg_t1 = nc.dram_tensor("dbg_t1", [NT, R1], F32, kind="ExternalOutput")
        dbg_ad1 = nc.dram_tensor("dbg_ad1", [P, NG], F32,
                                 kind="ExternalOutput")
        dbg_g1 = nc.dram_tensor("dbg_g1", [P, int(Kg[0]) * R1], F32,
                                kind="ExternalOutput")
        dbg_w = nc.dram_tensor("dbg_w", [P, int(Kg[0])], F32,
                               kind="ExternalOutput")
        dbg_o1 = nc.dram_tensor("dbg_o1", [P, HID], F32,
                                kind="ExternalOutput")

    t1loc = nc.dram_tensor("t1loc", [NPC1, R1], F32)
    t2loc = nc.dram_tensor("t2loc", [NPC1, R2], F32)
    t1 = nc.dram_tensor("t1", [NT, R1], F32, addr_space="Shared")
    t2 = nc.dram_tensor("t2", [NT, R2], F32, addr_space="Shared")
    rg = [list(range(NCORES))]

    with tile.TileContext(nc) as tc:
        with (
            tc.tile_pool(name="const", bufs=1) as cpool,
            tc.tile_pool(name="xt", bufs=2) as xpool,
            tc.tile_pool(name="ps1", bufs=2, space="PSUM") as ps1,
            tc.tile_pool(name="pst", bufs=2, space="PSUM") as pst,
            tc.tile_pool(name="ps2", bufs=2, space="PSUM") as ps2,
            tc.tile_pool(name="work", bufs=2) as wp,
            tc.tile_pool(name="big", bufs=2) as bigp,
            tc.tile_pool(name="mbuf", bufs=1) as mp,
        ):
            # ---- resident constants
            w1t4 = cpool.tile([P, 4 * R1], F32, tag="w1t4")
            nc.sync.dma_start(
                out=w1t4[:].rearrange("p (c r) -> p c r", r=R1),
                in_=w1e_d[:].rearrange("(c p) r -> p c r", p=P))
            w1t = [w1t4[:, cc * R1:(cc + 1) * R1] for cc in range(4)]
            w2t = cpool.tile([HID, R2], F32, tag="w2t")
            nc.sync.dma_start(out=w2t[:], in_=w2e_d[:])
            b1t = cpool.tile([P, HID], F32, tag="b1t")
            nc.sync.dma_start(out=b1t[:], in_=b1r_d[:])
            b2t = cpool.tile([P, NCLS], F32, tag="b2t")
            nc.sync.dma_start(out=b2t[:], in_=b2r_d[:])
            idxt = cpool.tile([P, S], mybir.dt.int32, tag="idxt")
            nc.sync.dma_start(out=idxt[:], in_=idx_d[:])
            ident = cpool.tile([P, P], F32, tag="ident")
            make_identity(nc, ident[:])
            ad1 = cpool.tile([P, NG], F32, tag="ad1")
            ad2 = cpool.tile([P, NG], F32, tag="ad2")

            nc.sync.dma_start(out=t1loc[NPC:NPC + 1, :], in_=dum1_d[:])
            nc.sync.dma_start(out=t2loc[NPC:NPC + 1, :], in_=dum2_d[:])

            # ---- phase A: warm-up matmul observes w1t4's DMA tick so the
            # first real matmul carries only its xt wait (LW allows 1 wait)
            pwarm = pst.tile([R1, 1], F32, tag="pwarm")
            nc.tensor.matmul(pwarm[:], lhsT=w1t4[:, 0:R1],
                             rhs=w1t4[:, 0:1], start=True, stop=True)

            prev_copy = None
            for g in range(NG):
                xt4 = xpool.tile([P, 4 * P], F32, tag="xt4")
                nc.gpsimd.dma_start(
                    out=xt4[:].rearrange("p (c n) -> p c n", n=P),
                    in_=xT[:, g * P:(g + 1) * P].rearrange(
                        "(c p) n -> p c n", p=P))
                ph = ps1.tile([P, R1], F32, tag="ph1")
                mms = []
                for cc in range(4):
                    mms.append(nc.tensor.matmul(
                        ph[:], lhsT=xt4[:, cc * P:(cc + 1) * P],
                        rhs=w1t[cc], start=(cc == 0), stop=(cc == 3)))
                # hand the PSUM WAR tick to cc=3 (free wait slot): walrus
                # allows only ONE sync wait on a Matmult's LW stage, and
                # cc=0 already carries the xt DMA wait.
                if prev_copy is not None:
                    tile.add_dep_helper(
                        mms[3].ins, prev_copy.ins,
                        reason="psum WAR tick via cc3")
                tt = wp.tile([P, R1], F32, tag="tt1")
                prev_copy = nc.scalar.copy(out=tt[:], in_=ph[:])
                nc.sync.dma_start(
                    out=t1loc[g * P:(g + 1) * P, :], in_=tt[:])

            nc.gpsimd.collective_compute(
                "AllGather", mybir.AluOpType.bypass, replica_groups=rg,
                ins=[t1loc[:]], outs=[t1[:]])
            # alpha_dst columns for my own nodes, one strided DMA
            nc.gpsimd.dma_start(
                out=ad1[:],
                in_=t1loc[0:NPC, R1 - 1:R1].rearrange(
                    "(g p) o -> p (g o)", p=P))
            if DEBUG_DUMPS:
                nc.sync.dma_start(out=dbg_t1[:], in_=t1[:])
                nc.gpsimd.dma_start(out=dbg_ad1[:], in_=ad1[:])

            # ---- phase B: layer-1 edge aggregation, build local table2
            for g in range(NG):
                K = int(Kg[g])
                o = int(offs[g])
                G1 = bigp.tile([P, Kmax * R1], F32, tag="G1")
                nc.sync.dma_start(out=G1[:, 0:R1],
                                  in_=t1loc[g * P:(g + 1) * P, :])
                for k in range(1, K):
                    nc.gpsimd.indirect_dma_start(
                        out=G1[:, k * R1:(k + 1) * R1],
                        out_offset=None, in_=t1[:],
                        in_offset=bass.IndirectOffsetOnAxis(
                            ap=idxt[:, o + k:o + k + 1], axis=0))
                g3 = G1[:, :K * R1].rearrange("p (k r) -> p k r", r=R1)

                e = wp.tile([P, Kmax], F32, tag="e1")
                nc.vector.tensor_scalar_add(
                    e[:, :K], g3[:, :, HID:HID + 1], ad1[:, g:g + 1])
                nc.vector.scalar_tensor_tensor(
                    out=e[:, :K], in0=e[:, :K], scalar=NEG_SLOPE,
                    in1=e[:, :K], op0=Alu.mult, op1=Alu.max)
                w = wp.tile([P, Kmax], F32, tag="w1")
                nc.scalar.activation(w[:, :K], e[:, :K], Act.Exp)
                den = wp.tile([P, 1], F32, tag="den1")
                nc.vector.tensor_reduce(
                    out=den[:], in_=w[:, :K], axis=X, op=Alu.add)
                nc.vector.tensor_scalar_add(den[:], den[:], 1e-16)
                rc = wp.tile([P, 1], F32, tag="rc1")
                nc.vector.reciprocal(rc[:], den[:])

                M = mp.tile([P, Kmax * HID], F32, tag="M1")
                m3 = M[:, :K * HID].rearrange("p (k r) -> p k r", r=HID)
                nc.vector.tensor_tensor(
                    out=m3, in0=g3[:, :, :HID],
                    in1=w[:, :K].unsqueeze(2).to_broadcast([P, K, HID]),
                    op=Alu.mult)
                red = wp.tile([P, HID], F32, tag="red1")
                nc.vector.tensor_reduce(
                    out=red[:],
                    in_=M[:, :K * HID].rearrange("p (k r) -> p r k", r=HID),
                    axis=X, op=Alu.add)
                o1 = wp.tile([P, HID], F32, tag="o1")
                nc.vector.scalar_tensor_tensor(
                    out=o1[:], in0=red[:], scalar=rc[:], in1=b1t[:],
                    op0=Alu.mult, op1=Alu.add)
                if DEBUG_DUMPS and g == 0:
                    nc.sync.dma_start(out=dbg_g1[:], in_=G1[:, :K * R1])
                    nc.sync.dma_start(out=dbg_w[:], in_=w[:, :K])
                    nc.sync.dma_start(out=dbg_o1[:], in_=o1[:])

                # ELU(x) = relu(x) + exp(min(x,0)) - 1
                tmin = wp.tile([P, HID], F32, tag="tmin")
                nc.vector.tensor_scalar_min(tmin[:], o1[:], 0.0)
                texp = wp.tile([P, HID], F32, tag="texp")
                nc.scalar.activation(texp[:], tmin[:], Act.Exp)
                trelu = wp.tile([P, HID], F32, tag="trelu")
                nc.vector.tensor_scalar_max(trelu[:], o1[:], 0.0)
                h1e = wp.tile([P, HID], F32, tag="h1e")
                nc.vector.scalar_tensor_tensor(
                    out=h1e[:], in0=texp[:], scalar=-1.0, in1=trelu[:],
                    op0=Alu.add, op1=Alu.add)

                # h2 = h1e @ W2e  (transpose h1e, contract over HID)
                pt = pst.tile([HID, P], F32, tag="pt")
                nc.tensor.transpose(pt[:], h1e[:], ident[:])
                h1eT = wp.tile([HID, P], F32, tag="h1eT")
                nc.vector.tensor_copy(h1eT[:], pt[:])
                ph2 = ps2.tile([P, R2], F32, tag="ph2")
                nc.tensor.matmul(
                    ph2[:], lhsT=h1eT[:], rhs=w2t[:], start=True, stop=True)
                t2t = wp.tile([P, R2], F32, tag="t2t")
                nc.vector.tensor_copy(t2t[:], ph2[:])
                nc.sync.dma_start(
                    out=t2loc[g * P:(g + 1) * P, :], in_=t2t[:])

            nc.gpsimd.collective_compute(
                "AllGather", mybir.AluOpType.bypass, replica_groups=rg,
                ins=[t2loc[:]], outs=[t2[:]])
            nc.gpsimd.dma_start(
                out=ad2[:],
                in_=t2loc[0:NPC, NCLS + 1:NCLS + 2].rearrange(
                    "(g p) o -> p (g o)", p=P))

            # ---- phase C: layer-2 edge aggregation + log_softmax
            for g in range(NG):
                K = int(Kg[g])
                o = int(offs[g])
                G2 = bigp.tile([P, Kmax * R2], F32, tag="G2")
                nc.sync.dma_start(out=G2[:, 0:R2],
                                  in_=t2loc[g * P:(g + 1) * P, :])
                for k in range(1, K):
                    nc.gpsimd.indirect_dma_start(
                        out=G2[:, k * R2:(k + 1) * R2],
                        out_offset=None, in_=t2[:],
                        in_offset=bass.IndirectOffsetOnAxis(
                            ap=idxt[:, o + k:o + k + 1], axis=0))
                q3 = G2[:, :K * R2].rearrange("p (k r) -> p k r", r=R2)

                e = wp.tile([P, Kmax], F32, tag="e2")
                nc.vector.tensor_scalar_add(
                    e[:, :K], q3[:, :, NCLS:NCLS + 1], ad2[:, g:g + 1])
                nc.vector.scalar_tensor_tensor(
                    out=e[:, :K], in0=e[:, :K], scalar=NEG_SLOPE,
                    in1=e[:, :K], op0=Alu.mult, op1=Alu.max)
                w = wp.tile([P, Kmax], F32, tag="w2")
                nc.scalar.activation(w[:, :K], e[:, :K], Act.Exp)
                den = wp.tile([P, 1], F32, tag="den2")
                nc.vector.tensor_reduce(
                    out=den[:], in_=w[:, :K], axis=X, op=Alu.add)
                nc.vector.tensor_scalar_add(den[:], den[:], 1e-16)
                rc = wp.tile([P, 1], F32, tag="rc2")
                nc.vector.reciprocal(rc[:], den[:])

                M = mp.tile([P, Kmax * NCLS], F32, tag="M2")
                m3 = M[:, :K * NCLS].rearrange("p (k r) -> p k r", r=NCLS)
                nc.vector.tensor_tensor(
                    out=m3, in0=q3[:, :, :NCLS],
                    in1=w[:, :K].unsqueeze(2).to_broadcast([P, K, NCLS]),
                    op=Alu.mult)
                red = wp.tile([P, NCLS], F32, tag="red2")
                nc.vector.tensor_reduce(
                    out=red[:],
                    in_=M[:, :K * NCLS].rearrange("p (k r) -> p r k", r=NCLS),
                    axis=X, op=Alu.add)
                logits = wp.tile([P, NCLS], F32, tag="logits")
                nc.vector.scalar_tensor_tensor(
                    out=logits[:], in0=red[:], scalar=rc[:], in1=b2t[:],
                    op0=Alu.mult, op1=Alu.add)

                # log_softmax
                mx = wp.tile([P, 1], F32, tag="mx")
                nc.vector.tensor_reduce(
                    out=mx[:], in_=logits[:], axis=X, op=Alu.max)
                sh = wp.tile([P, NCLS], F32, tag="sh")
                nc.vector.tensor_scalar_sub(sh[:], logits[:], mx[:])
                es = wp.tile([P, NCLS], F32, tag="es")
                nc.scalar.activation(es[:], sh[:], Act.Exp)
                ssum = wp.tile([P, 1], F32, tag="ssum")
                nc.vector.tensor_reduce(
                    out=ssum[:], in_=es[:], axis=X, op=Alu.add)
                lg = wp.tile([P, 1], F32, tag="lg")
                nc.scalar.activation(lg[:], ssum[:], Act.Ln)
                fin = wp.tile([P, NCLS], F32, tag="fin")
                nc.vector.tensor_scalar_sub(fin[:], sh[:], lg[:])
                nc.sync.dma_start(
                    out=outp[g * P:(g + 1) * P, :], in_=fin[:])

    if compile_module:
        nc.compile()
    return nc


# ------------------------------------------------------------------- runner
_CACHE = {}


def kernel_with_results(inputs, trace=False):
    in_maps, Kg, offs, S, order = preprocess(**inputs)
    key = (tuple(Kg.tolist()), S)
    if key not in _CACHE:
        _CACHE[key] = build_program(Kg, offs, S)
    nc = _CACHE[key]
    from concourse.bass_utils import run_bass_kernel_spmd
    res = run_bass_kernel_spmd(nc, in_maps, list(range(NCORES)), trace=trace)
    out = np.empty((N, NCLS), dtype=np.float32)
    for c in range(NCORES):
        out[order[c::NCORES]] = res.results[c]["out"][:NREAL]
    return out, res


def kernel(**inputs):
    out, _ = kernel_with_results(inputs, trace=False)
    return out


# revision 23
# speedup vs baseline: 1.3518x; 1.0104x over previous
"""Two-layer GAT (GATConv heads=1, PyG-style) on 8 Trainium2 NeuronCores.

Strategy (matches sharding_hint): nodes are degree-sorted and dealt
round-robin to the 8 cores (so every core sees the same degree profile,
letting one SPMD program with a fixed per-group ELL K-schedule serve all
cores). Edges are partitioned by destination node. Each core computes the
feature transform for its own node slice, the per-node tables
[h | alpha_src (| alpha_dst)] are AllGathered so each core holds the full
table in local HBM, and edge aggregation is done with multi-index indirect
DMA gathers (ELL layout: node = SBUF partition, edge slot = free dim)
followed by a segment softmax + weighted reduction on the vector/scalar
engines. Layer 2 repeats the pattern with a second AllGather.
"""
import sys

sys.path.insert(0, "/opt/trn_rl_repo")

import numpy as np

# ---------------------------------------------------------------- constants
N = 50000        # nodes
F_IN = 512       # input features
HID = 64         # layer-1 out features
NCLS = 40        # classes
NEG_SLOPE = 0.2

NCORES = 8
P = 128                       # SBUF partitions
NG = 49                       # node groups per core
NPC = NG * P                  # node slots per core (6272; 6250 real)
NREAL = N // NCORES           # real nodes per core (6250)
DEBUG_DUMPS = False           # add intermediate ExternalOutputs (debug only)
NPC1 = NPC + 1                # slice rows incl per-core dummy row
NT = NCORES * NPC1            # global permuted table rows (50184)
DUMMY = NPC                   # dummy row index = core-0's dummy slot
ALPHA_PAD = -60.0

R1 = HID + 2                  # table1 row: [h(64) | a_src | a_dst] = 66
R2 = 44                       # table2 row: [h2(40) | a_src | a_dst | pad] = 44

F32 = None  # filled after imports


def _import_bass():
    global bass, bacc, mybir, tile, F32
    import concourse.bass as bass
    import concourse.bacc as bacc
    import concourse.mybir as mybir
    import concourse.tile as tile
    F32 = mybir.dt.float32
    return bass, mybir, tile


# ---------------------------------------------------------------- host prep
def preprocess(x, edge_index, W1, a_src1, a_dst1, b1, W2, a_src2, a_dst2, b2):
    """Degree-sort nodes, deal round-robin to cores, build ELL edge arrays."""
    src = np.asarray(edge_index[0], dtype=np.int64)
    dst = np.asarray(edge_index[1], dtype=np.int64)
    loops = np.arange(N, dtype=np.int64)
    src = np.concatenate([loops, src])
    dst = np.concatenate([loops, dst])

    deg = np.bincount(dst, minlength=N)          # in-degree incl self-loop
    order = np.argsort(-deg, kind="stable")      # order[r] = node of rank r
    rank = np.empty(N, dtype=np.int64)
    rank[order] = np.arange(N)

    # global permuted-table position of each node
    ptab = (rank % NCORES) * NPC1 + rank // NCORES

    # K schedule: group g holds ranks [1024g, 1024(g+1)); max degree is at
    # the first rank of the stripe (degrees sorted descending)
    deg_sorted = deg[order]
    Kg = deg_sorted[np.arange(NG) * (NCORES * P)].astype(np.int64)
    offs = np.concatenate([[0], np.cumsum(Kg)])
    S = int(offs[-1])

    # ELL fill: edge (s -> d) goes to core/slot of d, column offs[g] + k
    # where k = index of the edge within d's in-edge list.
    eorder = np.argsort(dst, kind="stable")
    sdst = dst[eorder]
    ssrc = src[eorder]
    starts = np.cumsum(deg) - deg                # first edge index per dst
    ke = np.arange(sdst.shape[0]) - starts[sdst]

    rd = rank[sdst]
    c_e = rd % NCORES
    pos = rd // NCORES
    g_e = pos // P
    p_e = pos % P
    col = offs[g_e] + ke

    idx_arr = np.full((NCORES, P, S), DUMMY, dtype=np.int32)
    flat = (c_e * P + p_e) * S + col
    idx_arr.reshape(-1)[flat] = ptab[ssrc].astype(np.int32)

    # per-core transposed x slices (node-permuted, zero-padded)
    x = np.asarray(x, dtype=np.float32)
    xT = []
    for c in range(NCORES):
        xc = np.zeros((NPC, F_IN), dtype=np.float32)
        xc[:NREAL] = x[order[c::NCORES]]
        xT.append(np.ascontiguousarray(xc.T))

    # extended weights: fold alpha projections into the matmul
    W1 = np.asarray(W1, np.float32)
    W2 = np.asarray(W2, np.float32)
    w1e = np.concatenate(
        [W1, (W1 @ np.asarray(a_src1, np.float32))[:, None],
         (W1 @ np.asarray(a_dst1, np.float32))[:, None]], axis=1)  # [512, 66]
    w2e = np.zeros((HID, R2), dtype=np.float32)
    w2e[:, :NCLS] = W2
    w2e[:, NCLS] = W2 @ np.asarray(a_src2, np.float32)
    w2e[:, NCLS + 1] = W2 @ np.asarray(a_dst2, np.float32)

    b1r = np.ascontiguousarray(
        np.broadcast_to(np.asarray(b1, np.float32), (P, HID)))
    b2r = np.ascontiguousarray(
        np.broadcast_to(np.asarray(b2, np.float32), (P, NCLS)))

    dum1 = np.zeros((1, R1), dtype=np.float32)
    dum1[0, HID] = ALPHA_PAD
    dum2 = np.zeros((1, R2), dtype=np.float32)
    dum2[0, NCLS] = ALPHA_PAD

    in_maps = [
        {"xT": xT[c], "w1e": w1e, "w2e": w2e, "b1r": b1r, "b2r": b2r,
         "idx": idx_arr[c], "dum1": dum1, "dum2": dum2}
        for c in range(NCORES)
    ]
    return in_maps, Kg, offs, S, order


# ------------------------------------------------------------- bass program
def build_program(Kg, offs, S, compile_module=True):
    bass, mybir, tile = _import_bass()
    from concourse.masks import make_identity
    Alu = mybir.AluOpType
    Act = mybir.ActivationFunctionType
    X = mybir.AxisListType.X
    Kmax = int(max(Kg))

    nc = bacc.Bacc("TRN2", num_devices=NCORES)

    xT = nc.dram_tensor("xT", [F_IN, NPC], F32, kind="ExternalInput")
    w1e_d = nc.dram_tensor("w1e", [F_IN, R1], F32, kind="ExternalInput")
    w2e_d = nc.dram_tensor("w2e", [HID, R2], F32, kind="ExternalInput")
    b1r_d = nc.dram_tensor("b1r", [P, HID], F32, kind="ExternalInput")
    b2r_d = nc.dram_tensor("b2r", [P, NCLS], F32, kind="ExternalInput")
    idx_d = nc.dram_tensor("idx", [P, S], mybir.dt.int32, kind="ExternalInput")
    dum1_d = nc.dram_tensor("dum1", [1, R1], F32, kind="ExternalInput")
    dum2_d = nc.dram_tensor("dum2", [1, R2], F32, kind="ExternalInput")
    outp = nc.dram_tensor("out", [NPC, NCLS], F32, kind="ExternalOutput")
    if DEBUG_DUMPS:
        dbg_t1 = nc.dram_tensor("dbg_t1", [NT, R1], F32, kind="ExternalOutput")
        dbg_ad1 = nc.dram_tensor("dbg_ad1", [P, NG], F32,
                                 kind="ExternalOutput")
        dbg_g1 = nc.dram_tensor("dbg_g1", [P, int(Kg[0]) * R1], F32,
                                kind="ExternalOutput")
        dbg_w = nc.dram_tensor("dbg_w", [P, int(Kg[0])], F32,
                               kind="ExternalOutput")
        dbg_o1 = nc.dram_tensor("dbg_o1", [P, HID], F32,
                                kind="ExternalOutput")

    t1loc = nc.dram_tensor("t1loc", [NPC1, R1], F32)
    t2loc = nc.dram_tensor("t2loc", [NPC1, R2], F32)
    t1 = nc.dram_tensor("t1", [NT, R1], F32, addr_space="Shared")
    t2 = nc.dram_tensor("t2", [NT, R2], F32, addr_space="Shared")
    rg = [list(range(NCORES))]

    with tile.TileContext(nc) as tc:
        with (
            tc.tile_pool(name="const", bufs=1) as cpool,
            tc.tile_pool(name="xt", bufs=2) as xpool,
            tc.tile_pool(name="ps1", bufs=2, space="PSUM") as ps1,
            tc.tile_pool(name="pst", bufs=2, space="PSUM") as pst,
            tc.tile_pool(name="ps2", bufs=2, space="PSUM") as ps2,
            tc.tile_pool(name="work", bufs=2) as wp,
            tc.tile_pool(name="big", bufs=2) as bigp,
            tc.tile_pool(name="mbuf", bufs=1) as mp,
        ):
            # ---- resident constants
            w1t4 = cpool.tile([P, 4 * R1], F32, tag="w1t4")
            nc.sync.dma_start(
                out=w1t4[:].rearrange("p (c r) -> p c r", r=R1),
                in_=w1e_d[:].rearrange("(c p) r -> p c r", p=P))
            w1t = [w1t4[:, cc * R1:(cc + 1) * R1] for cc in range(4)]
            w2t = cpool.tile([HID, R2], F32, tag="w2t")
            nc.sync.dma_start(out=w2t[:], in_=w2e_d[:])
            b1t = cpool.tile([P, HID], F32, tag="b1t")
            nc.sync.dma_start(out=b1t[:], in_=b1r_d[:])
            b2t = cpool.tile([P, NCLS], F32, tag="b2t")
            nc.sync.dma_start(out=b2t[:], in_=b2r_d[:])
            idxt = cpool.tile([P, S], mybir.dt.int32, tag="idxt")
            nc.sync.dma_start(out=idxt[:], in_=idx_d[:])
            ident = cpool.tile([P, P], F32, tag="ident")
            make_identity(nc, ident[:])
            ad1 = cpool.tile([P, NG], F32, tag="ad1")
            ad2 = cpool.tile([P, NG], F32, tag="ad2")

            nc.sync.dma_start(out=t1loc[NPC:NPC + 1, :], in_=dum1_d[:])
            nc.sync.dma_start(out=t2loc[NPC:NPC + 1, :], in_=dum2_d[:])

            # ---- phase A: warm-up matmul observes w1t4's DMA tick so the
            # first real matmul carries only its xt wait (LW allows 1 wait)
            pwarm = pst.tile([R1, 1], F32, tag="pwarm")
            nc.tensor.matmul(pwarm[:], lhsT=w1t4[:, 0:R1],
                             rhs=w1t4[:, 0:1], start=True, stop=True)

            prev_copy = None
            for g in range(NG):
                xt4 = xpool.tile([P, 4 * P], F32, tag="xt4")
                nc.gpsimd.dma_start(
                    out=xt4[:].rearrange("p (c n) -> p c n", n=P),
                    in_=xT[:, g * P:(g + 1) * P].rearrange(
                        "(c p) n -> p c n", p=P))
                ph = ps1.tile([P, R1], F32, tag="ph1")
                mms = []
                for cc in range(4):
                    mms.append(nc.tensor.matmul(
                        ph[:], lhsT=xt4[:, cc * P:(cc + 1) * P],
                        rhs=w1t[cc], start=(cc == 0), stop=(cc == 3)))
                # hand the PSUM WAR tick to cc=3 (free wait slot): walrus
                # allows only ONE sync wait on a Matmult's LW stage, and
                # cc=0 already carries the xt DMA wait.
                if prev_copy is not None:
                    tile.add_dep_helper(
                        mms[3].ins, prev_copy.ins,
                        reason="psum WAR tick via cc3")
                tt = wp.tile([P, R1], F32, tag="tt1")
                prev_copy = nc.scalar.copy(out=tt[:], in_=ph[:])
                nc.sync.dma_start(
                    out=t1loc[g * P:(g + 1) * P, :], in_=tt[:])

            nc.gpsimd.collective_compute(
                "AllGather", mybir.AluOpType.bypass, replica_groups=rg,
                ins=[t1loc[:]], outs=[t1[:]])
            # alpha_dst columns for my own nodes, one strided DMA
            nc.gpsimd.dma_start(
                out=ad1[:],
                in_=t1loc[0:NPC, R1 - 1:R1].rearrange(
                    "(g p) o -> p (g o)", p=P))
            if DEBUG_DUMPS:
                nc.sync.dma_start(out=dbg_t1[:], in_=t1[:])
                nc.gpsimd.dma_start(out=dbg_ad1[:], in_=ad1[:])

            # ---- phase B: layer-1 edge aggregation, build local table2
            for g in range(NG):
                K = int(Kg[g])
                o = int(offs[g])
                G1 = bigp.tile([P, Kmax * R1], F32, tag="G1")
                nc.sync.dma_start(out=G1[:, 0:R1],
                                  in_=t1loc[g * P:(g + 1) * P, :])
                for k in range(1, K):
                    nc.gpsimd.indirect_dma_start(
                        out=G1[:, k * R1:(k + 1) * R1],
                        out_offset=None, in_=t1[:],
                        in_offset=bass.IndirectOffsetOnAxis(
                            ap=idxt[:, o + k:o + k + 1], axis=0))
                g3 = G1[:, :K * R1].rearrange("p (k r) -> p k r", r=R1)

                e = wp.tile([P, Kmax], F32, tag="e1")
                nc.vector.tensor_scalar_add(
                    e[:, :K], g3[:, :, HID:HID + 1], ad1[:, g:g + 1])
                nc.vector.scalar_tensor_tensor(
                    out=e[:, :K], in0=e[:, :K], scalar=NEG_SLOPE,
                    in1=e[:, :K], op0=Alu.mult, op1=Alu.max)
                w = wp.tile([P, Kmax], F32, tag="w1")
                nc.scalar.activation(w[:, :K], e[:, :K], Act.Exp)
                den = wp.tile([P, 1], F32, tag="den1")
                nc.vector.tensor_reduce(
                    out=den[:], in_=w[:, :K], axis=X, op=Alu.add)
                nc.vector.tensor_scalar_add(den[:], den[:], 1e-16)
                rc = wp.tile([P, 1], F32, tag="rc1")
                nc.vector.reciprocal(rc[:], den[:])

                M = mp.tile([P, Kmax * HID], F32, tag="M1")
                m3 = M[:, :K * HID].rearrange("p (k r) -> p k r", r=HID)
                nc.vector.tensor_tensor(
                    out=m3, in0=g3[:, :, :HID],
                    in1=w[:, :K].unsqueeze(2).to_broadcast([P, K, HID]),
                    op=Alu.mult)
                red = wp.tile([P, HID], F32, tag="red1")
                nc.vector.tensor_reduce(
                    out=red[:],
                    in_=M[:, :K * HID].rearrange("p (k r) -> p r k", r=HID),
                    axis=X, op=Alu.add)
                o1 = wp.tile([P, HID], F32, tag="o1")
                nc.vector.scalar_tensor_tensor(
                    out=o1[:], in0=red[:], scalar=rc[:], in1=b1t[:],
                    op0=Alu.mult, op1=Alu.add)
                if DEBUG_DUMPS and g == 0:
                    nc.sync.dma_start(out=dbg_g1[:], in_=G1[:, :K * R1])
                    nc.sync.dma_start(out=dbg_w[:], in_=w[:, :K])
                    nc.sync.dma_start(out=dbg_o1[:], in_=o1[:])

                # ELU(x) = relu(x) + exp(min(x,0)) - 1
                tmin = wp.tile([P, HID], F32, tag="tmin")
                nc.vector.tensor_scalar_min(tmin[:], o1[:], 0.0)
                texp = wp.tile([P, HID], F32, tag="texp")
                nc.scalar.activation(texp[:], tmin[:], Act.Exp)
                trelu = wp.tile([P, HID], F32, tag="trelu")
                nc.vector.tensor_scalar_max(trelu[:], o1[:], 0.0)
                h1e = wp.tile([P, HID], F32, tag="h1e")
                nc.vector.scalar_tensor_tensor(
                    out=h1e[:], in0=texp[:], scalar=-1.0, in1=trelu[:],
                    op0=Alu.add, op1=Alu.add)

                # h2 = h1e @ W2e  (transpose h1e, contract over HID)
                pt = pst.tile([HID, P], F32, tag="pt")
                nc.tensor.transpose(pt[:], h1e[:], ident[:])
                h1eT = wp.tile([HID, P], F32, tag="h1eT")
                nc.vector.tensor_copy(h1eT[:], pt[:])
                ph2 = ps2.tile([P, R2], F32, tag="ph2")
                nc.tensor.matmul(
                    ph2[:], lhsT=h1eT[:], rhs=w2t[:], start=True, stop=True)
                t2t = wp.tile([P, R2], F32, tag="t2t")
                nc.vector.tensor_copy(t2t[:], ph2[:])
                nc.sync.dma_start(
                    out=t2loc[g * P:(g + 1) * P, :], in_=t2t[:])

            nc.gpsimd.collective_compute(
                "AllGather", mybir.AluOpType.bypass, replica_groups=rg,
                ins=[t2loc[:]], outs=[t2[:]])
            nc.gpsimd.dma_start(
                out=ad2[:],
                in_=t2loc[0:NPC, NCLS + 1:NCLS + 2].rearrange(
                    "(g p) o -> p (g o)", p=P))

            # ---- phase C: layer-2 edge aggregation + log_softmax
            for g in range(NG):
                K = int(Kg[g])
                o = int(offs[g])
                G2 = bigp.tile([P, Kmax * R2], F32, tag="G2")
                nc.sync.dma_start(out=G2[:, 0:R2],
                                  in_=t2loc[g * P:(g + 1) * P, :])
                for k in range(1, K):
                    nc.gpsimd.indirect_dma_start(
                        out=G2[:, k * R2:(k + 1) * R2],
                        out_offset=None, in_=t2[:],
                        in_offset=bass.IndirectOffsetOnAxis(
                            ap=idxt[:, o + k:o + k + 1], axis=0))
                q3 = G2[:, :K * R2].rearrange("p (k r) -> p k r", r=R2)

                e = wp.tile([P, Kmax], F32, tag="e2")
                nc.vector.tensor_scalar_add(
                    e[:, :K], q3[:, :, NCLS:NCLS + 1], ad2[:, g:g + 1])
                nc.vector.scalar_tensor_tensor(
                    out=e[:, :K], in0=e[:, :K], scalar=NEG_SLOPE,
                    in1=e[:, :K], op0=Alu.mult, op1=Alu.max)
                w = wp.tile([P, Kmax], F32, tag="w2")
                nc.scalar.activation(w[:, :K], e[:, :K], Act.Exp)
                den = wp.tile([P, 1], F32, tag="den2")
                nc.vector.tensor_reduce(
                    out=den[:], in_=w[:, :K], axis=X, op=Alu.add)
                nc.vector.tensor_scalar_add(den[:], den[:], 1e-16)
                rc = wp.tile([P, 1], F32, tag="rc2")
                nc.vector.reciprocal(rc[:], den[:])

                M = mp.tile([P, Kmax * NCLS], F32, tag="M2")
                m3 = M[:, :K * NCLS].rearrange("p (k r) -> p k r", r=NCLS)
                nc.vector.tensor_tensor(
                    out=m3, in0=q3[:, :, :NCLS],
                    in1=w[:, :K].unsqueeze(2).to_broadcast([P, K, NCLS]),
                    op=Alu.mult)
                red = wp.tile([P, NCLS], F32, tag="red2")
                nc.vector.tensor_reduce(
                    out=red[:],
                    in_=M[:, :K * NCLS].rearrange("p (k r) -> p r k", r=NCLS),
                    axis=X, op=Alu.add)
                logits = wp.tile([P, NCLS], F32, tag="logits")
                nc.vector.scalar_tensor_tensor(
                    out=logits[:], in0=red[:], scalar=rc[:], in1=b2t[:],
                    op0=Alu.mult, op1=Alu.add)

                # log_softmax
                mx = wp.tile([P, 1], F32, tag="mx")
                nc.vector.tensor_reduce(
                    out=mx[:], in_=logits[:], axis=X, op=Alu.max)
                sh = wp.tile([P, NCLS], F32, tag="sh")
                nc.vector.tensor_scalar_sub(sh[:], logits[:], mx[:])
                es = wp.tile([P, NCLS], F32, tag="es")
                nc.scalar.activation(es[:], sh[:], Act.Exp)
                ssum = wp.tile([P, 1], F32, tag="ssum")
                nc.vector.tensor_reduce(
                    out=ssum[:], in_=es[:], axis=X, op=Alu.add)
                lg = wp.tile([P, 1], F32, tag="lg")
                nc.scalar.activation(lg[:], ssum[:], Act.Ln)
                fin = wp.tile([P, NCLS], F32, tag="fin")
                nc.vector.tensor_scalar_sub(fin[:], sh[:], lg[:])
                nc.sync.dma_start(
                    out=outp[g * P:(g + 1) * P, :], in_=fin[:])

    if compile_module:
        nc.compile()
    return nc


# ------------------------------------------------------------------- runner
_CACHE = {}


def kernel_with_results(inputs, trace=False):
    in_maps, Kg, offs, S, order = preprocess(**inputs)
    key = (tuple(Kg.tolist()), S)
    if key not in _CACHE:
        _CACHE[key] = build_program(Kg, offs, S)
    nc = _CACHE[key]
    from concourse.bass_utils import run_bass_kernel_spmd
    res = run_bass_kernel_spmd(nc, in_maps, list(range(NCORES)), trace=trace)
    out = np.empty((N, NCLS), dtype=np.float32)
    for c in range(NCORES):
        out[order[c::NCORES]] = res.results[c]["out"][:NREAL]
    return out, res


def kernel(**inputs):
    out, _ = kernel_with_results(inputs, trace=False)
    return out
